# revision 2
# baseline (speedup 1.0000x reference)
"""GCNConv (PyG semantics) on 8 Trainium2 NeuronCores — scatter-add design.

out = D^-1/2 (A+I) D^-1/2 (x @ W.T) + b, dst-sharded across 8 cores.

Host: per core, sort nodes by core-local out-degree (desc). Level r covers
the sorted prefix of nodes with deg >= r; levels split into calls of <= 48
columns (6144 positions). Each node's edges map bijectively onto its levels'
calls (+ level-free overflow calls over the low-degree band). Per call,
every assigned accumulator row (dst*2 + replica) is unique (host solver);
unfilled positions scatter to trash rows.

Device, per core:
  Phase A: h' = (x @ W.T) * dinv[src] -> SBUF-resident fp16 table
           [128, KCOLS, 64] (sorted position j -> (j%128, j//128)).
  Phase B: per call (ordered by required h' column): dma_scatter_add of its
           position range into acc block (chain_index % 3). The 3 blocks
           are separate pre-zeroed DRAM tensors; call i waits on call
           i-3's DMA completion, so concurrent transfers always target
           distinct blocks and the CCE read-modify-write never races on a
           row (within-call rows are unique by construction).
  Phase C: read blocks back (only the used 64-elem half of each 128-elem
           row), merge the two replica sub-rows, scale by dinv[dst], add
           bias, write out shard (fp16; host casts to fp32).
"""

import numpy as np
from collections import defaultdict
from contextlib import ExitStack

import concourse.bacc as bacc
import concourse.bass as bass
import concourse.mybir as mybir
from concourse import bass_utils
from concourse.library_config import mlp

D = 64
N = 100000
NCORES = 8
SHARD = N // NCORES          # 12500
OUTROWS = 12544              # 128 * 98
OCOLS = OUTROWS // 128       # 98
ACCROWS = 2 * SHARD + 88     # 25088 rows per block; [25000,25088) = trash
TRASH0 = 2 * SHARD
ACOLS = ACCROWS // 128       # 196
NBLK = 3
CALLCOLS = 48

LAST_NC = None


def _wrap16(idx_flat):
    n = idx_flat.shape[0]
    out = idx_flat.reshape(n // 16, 16).T.astype(np.int16)
    return np.tile(out, (8, 1))


def _build_geometry(K):
    cols = [-(-int(k) // 128) for k in K]
    calls = []
    for r in range(len(K)):
        for a in range(0, cols[r], CALLCOLS):
            calls.append((a, min(CALLCOLS, cols[r] - a), r))
    band_lo = cols[2] if len(K) > 2 else 0
    for a in range(band_lo, cols[0], CALLCOLS):
        calls.append((a, min(CALLCOLS, cols[0] - a), -1))
    kcols = -(-cols[0] // 8) * 8
    return calls, kcols


def _solve_core(src_g, dst_l, K, calls, seed):
    """Assign each edge to (call, acc_row) with rows unique per call."""
    rng = np.random.default_rng(seed)
    E = src_g.shape[0]
    deg = np.bincount(src_g, minlength=N)
    order = np.argsort(-deg, kind="stable")
    pos_of = np.empty(N, np.int64)
    pos_of[order] = np.arange(N)

    prim = {}
    ovf_of_col = defaultdict(list)
    for i, (a, w, lev) in enumerate(calls):
        for cc in range(a, a + w):
            if lev < 0:
                ovf_of_col[cc].append(i)
            else:
                prim[(lev, cc)] = i

    p = pos_of[src_g]
    o = np.lexsort((rng.random(E), p))
    p_s = p[o]
    dst_s = dst_l[o].astype(np.int64)
    new = np.ones(E, bool)
    new[1:] = p_s[1:] != p_s[:-1]
    starts = np.nonzero(new)[0]
    grp_id = np.cumsum(new) - 1
    glen = np.diff(np.append(starts, E))
    pos_g = p_s[starts].copy()          # group -> position (mutable)
    grp_at_pos = {int(pos_g[g]): g for g in range(len(starts))}

    cid = np.full(E, -1, np.int64)
    row = np.full(E, -1, np.int64)
    slot_of = {}     # call*32768 + row -> edge
    callee = {}      # grp*256 + call -> edge

    def cand_calls(e):
        g = grp_id[e]
        c = int(pos_g[g]) // 128
        cl = [prim[(r, c)] for r in range(int(glen[g]))]
        return cl + ovf_of_col.get(c, [])

    def do_place(e, c2, r2):
        cid[e] = c2
        row[e] = r2
        slot_of[c2 * 32768 + r2] = e
        callee[int(grp_id[e]) * 256 + c2] = e

    def un_place(e):
        slot_of.pop(int(cid[e]) * 32768 + int(row[e]), None)
        callee.pop(int(grp_id[e]) * 256 + int(cid[e]), None)
        cid[e] = -1
        row[e] = -1

    def try_place(e, depth, vis):
        if e in vis:
            return False
        vis.add(e)
        g = int(grp_id[e])
        d2 = int(dst_s[e]) * 2
        cl = cand_calls(e)
        rng.shuffle(cl)
        for c2 in cl:
            if g * 256 + c2 in callee:
                continue
            for r2 in (d2, d2 + 1):
                if c2 * 32768 + r2 not in slot_of:
                    do_place(e, c2, r2)
                    return True
        if depth == 0:
            return False
        for c2 in cl:
            sib = callee.get(g * 256 + c2)
            if sib is not None:
                if sib == e:
                    continue
                sc, sr = int(cid[sib]), int(row[sib])
                free_r = None
                for r2 in (d2, d2 + 1):
                    occ = slot_of.get(c2 * 32768 + r2)
                    if occ is None or occ == sib:
                        free_r = r2
                        break
                if free_r is None:
                    continue
                un_place(sib)
                if c2 * 32768 + free_r in slot_of:
                    do_place(sib, sc, sr)
                    continue
                do_place(e, c2, free_r)
                if try_place(sib, depth - 1, vis):
                    return True
                un_place(e)
                do_place(sib, sc, sr)
                continue
            for r2 in (d2, d2 + 1):
                b = slot_of.get(c2 * 32768 + r2)
                if b is None:
                    do_place(e, c2, r2)
                    return True
                bc, br = int(cid[b]), int(row[b])
                un_place(b)
                do_place(e, c2, r2)
                if try_place(b, depth - 1, vis):
                    return True
                un_place(e)
                do_place(b, bc, br)
        return False

    order_e = np.arange(E)
    rng.shuffle(order_e)
    for e in order_e:
        try_place(int(e), 0, set())
    for depth in (1, 2, 3, 4):
        left = np.nonzero(cid < 0)[0]
        if left.size == 0:
            break
        for e in left:
            try_place(int(e), depth, set())

    # node-swap escape for the stubborn tail
    left = np.nonzero(cid < 0)[0]
    for e in left:
        e = int(e)
        if cid[e] >= 0:
            continue
        g = int(grp_id[e])
        dg = int(glen[g])
        lo = int(K[dg]) if dg < len(K) else 0
        hi = int(K[dg - 1])
        done = False
        for _ in range(500):
            v_pos = int(rng.integers(lo, hi))
            vg = grp_at_pos.get(v_pos)
            if vg is None or vg == g or int(glen[vg]) != dg:
                continue
            if v_pos // 128 == int(pos_g[g]) // 128:
                continue
            ues = list(range(int(starts[g]), int(starts[g]) + dg))
            ves = list(range(int(starts[vg]), int(starts[vg]) + dg))
            saved = [(int(cid[x]), int(row[x])) for x in ues + ves]
            for x in ues + ves:
                if cid[x] >= 0:
                    un_place(x)
            u_pos = int(pos_g[g])
            pos_g[g], pos_g[vg] = v_pos, u_pos
            grp_at_pos[v_pos], grp_at_pos[u_pos] = g, vg
            ok = True
            for x in ues + ves:
                if not try_place(x, 2, set()):
                    ok = False
                    break
            if ok:
                u_node = order[u_pos]
                v_node = order[v_pos]
                order[u_pos], order[v_pos] = v_node, u_node
                done = True
                break
            for x in ues + ves:
                if cid[x] >= 0:
                    un_place(x)
            pos_g[g], pos_g[vg] = u_pos, v_pos
            grp_at_pos[u_pos], grp_at_pos[v_pos] = g, vg
            for x, (sc, sr) in zip(ues + ves, saved):
                if sc >= 0:
                    do_place(x, sc, sr)
        if not done and cid[e] < 0:
            raise RuntimeError("node-swap failed for an edge")

    if int((cid < 0).sum()):
        raise RuntimeError("solver left edges unplaced")
    keys = cid * 32768 + row
    if len(np.unique(keys)) != E:
        raise RuntimeError("solver produced duplicate (call,row)")

    per_call = []
    for i, (a, w, lev) in enumerate(calls):
        n = 128 * w
        per_call.append(TRASH0 + (np.arange(n) % (ACCROWS - TRASH0)))
    # scatter positions: edge at table position pos_g[grp] + rank-in-group?
    # positions of edges within a group all equal the node position; the
    # edge's slot inside its call is node position - call base.
    for e in range(E):
        c2 = int(cid[e])
        a = calls[c2][0]
        pos = int(pos_g[int(grp_id[e])])
        per_call[c2][pos - a * 128] = int(row[e])
    return order, per_call


def _build_program(KCOLS, IDXC, sched):
    dt = mybir.dt
    AIT = KCOLS // 8
    NCALLS = len(sched)

    nc = bacc.Bacc("TRN2", target_bir_lowering=False, debug=False,
                   num_devices=NCORES)
    t_xT = nc.dram_tensor("xT", [D, KCOLS * 128], dt.float16,
                          kind="ExternalInput")
    t_WT = nc.dram_tensor("WT", [D, D], dt.float16, kind="ExternalInput")
    t_degT = nc.dram_tensor("degT", [128, KCOLS], dt.float32,
                            kind="ExternalInput")
    t_degD = nc.dram_tensor("degD", [128, OCOLS], dt.float32,
                            kind="ExternalInput")
    t_bBC = nc.dram_tensor("bBC", [128, D], dt.float32,
                           kind="ExternalInput")
    t_idx = nc.dram_tensor("idx", [128, IDXC], dt.int16,
                           kind="ExternalInput")
    t_acc = [nc.dram_tensor(f"acc{b}", [ACCROWS, 128], dt.float16,
                            kind="ExternalOutput") for b in range(NBLK)]
    t_out = nc.dram_tensor("out_s", [OUTROWS, D], dt.float16,
                           kind="ExternalOutput")

    cnt_blk = [0] * NBLK
    for i in range(NCALLS):
        cnt_blk[i % NBLK] += 1

    with ExitStack() as ctx:
        e = ctx.enter_context
        hp = e(nc.sbuf_tensor("hp", [128, KCOLS, D], dt.float16))
        scr = e(nc.sbuf_tensor("scr", [128, IDXC], dt.int16))
        xb = [e(nc.sbuf_tensor(f"xb{i}", [D, 1024], dt.float16))
              for i in range(4)]
        WTs = e(nc.sbuf_tensor("WTs", [D, D], dt.float16))
        degTs = e(nc.sbuf_tensor("degTs", [128, KCOLS], dt.float32))
        dinvTs = e(nc.sbuf_tensor("dinvTs", [128, KCOLS], dt.float32))
        degDs = e(nc.sbuf_tensor("degDs", [128, OCOLS], dt.float32))
        dinvDs = e(nc.sbuf_tensor("dinvDs", [128, OCOLS], dt.float32))
        bBCs = e(nc.sbuf_tensor("bBCs", [128, D], dt.float32))
        accm = e(nc.sbuf_tensor("accm", [128, OCOLS, D], dt.float32))
        psum = [e(nc.psum_tensor(f"ps{i}", [128, 512], dt.float32))
                for i in range(8)]

        sLD = e(nc.semaphore("sLD"))
        sDin = e(nc.semaphore("sDin"))
        sAx = [e(nc.semaphore(f"sAx{i}")) for i in range(4)]
        sAmm = e(nc.semaphore("sAmm"))
        sAh = e(nc.semaphore("sAh"))
        sDma = [e(nc.semaphore(f"sDma{b}")) for b in range(NBLK)]
        sRd = [e(nc.semaphore(f"sRd{b}")) for b in range(NBLK)]
        sMg = e(nc.semaphore("sMg"))
        sFin = e(nc.semaphore("sFin"))

        # aliased views over scr (idx dead by the time these are used)
        accT = [scr[:, b * ACOLS * D:(b + 1) * ACOLS * D]
                .bitcast(dt.float16).rearrange("p (c d) -> p c d", d=D)
                for b in range(2)]
        # outT aliases accT[0]'s bytes: block-2 data there is fully consumed
        # by the (sequential) vector merges before the bias op writes outT
        outT = scr[:, 0:OCOLS * D] \
            .bitcast(dt.float16).rearrange("p (c d) -> p c d", d=D)

        def bcast(ap, reps):
            return bass.AP(ap.tensor, ap.offset, list(ap.ap) + [[0, reps]])

        with nc.Block() as block:

            @block.sync
            def _(sync: bass.BassEngine):
                sync.dma_start(WTs[:], t_WT[:]).then_inc(sLD, 16)
                sync.dma_start(degTs[:], t_degT[:]).then_inc(sLD, 16)
                sync.dma_start(degDs[:], t_degD[:]).then_inc(sLD, 16)
                sync.dma_start(bBCs[:], t_bBC[:]).then_inc(sLD, 16)
                sync.dma_start(scr[:], t_idx[:]).then_inc(sLD, 16)
                for it in range(AIT):
                    if it >= 4:
                        sync.wait_ge(sAmm, it - 3)
                    sync.dma_start(
                        xb[it % 4][:],
                        t_xT[:, it * 1024:(it + 1) * 1024],
                    ).then_inc(sAx[it % 4], 16)
                for b in range(NBLK):
                    sync.wait_ge(sDma[b], cnt_blk[b] * 16)
                for b in range(NBLK):
                    if b >= 2:
                        sync.wait_ge(sMg, 1)
                    src = bass.AP(t_acc[b], 0,
                                  [[ACOLS * 128, 128], [128, ACOLS],
                                   [1, D]])
                    sync.dma_start(accT[b % 2], src).then_inc(sRd[b], 16)
                sync.wait_ge(sMg, 7)
                out3 = bass.AP(t_out, 0,
                               [[OCOLS * D, 128], [D, OCOLS], [1, D]])
                sync.dma_start(out3, outT).then_inc(sFin, 16)
                sync.wait_ge(sFin, 16)

            @block.tensor
            def _(tensor):
                tensor.wait_ge(sLD, 80)  # DMA completions may reorder
                for it in range(AIT):
                    tensor.wait_ge(sAx[it % 4], (it // 4 + 1) * 16)
                    if it >= 8:
                        tensor.wait_ge(sAh, it - 7)
                    for j in range(8):
                        ins = tensor.matmul(
                            psum[it % 8][:, j * D:(j + 1) * D],
                            xb[it % 4][:, j * 128:(j + 1) * 128],
                            WTs[:],
                            start=True, stop=True,
                        )
                    ins.then_inc(sAmm, 1)

            @block.vector
            def _(vector):
                vector.wait_ge(sLD, 80)  # DMA completions may reorder
                vector.reciprocal(dinvTs[:], degTs[:])
                vector.reciprocal(dinvDs[:], degDs[:]).then_inc(sDin, 1)
                vector.wait_ge(sDin, 2)
                for it in range(AIT):
                    vector.wait_ge(sAmm, it + 1)
                    ps3 = psum[it % 8][:].rearrange("p (c d) -> p c d", d=D)
                    dv = bcast(dinvTs[:, it * 8:(it + 1) * 8], D)
                    vector.tensor_tensor(
                        hp[:, it * 8:(it + 1) * 8, :], ps3, dv,
                        op=mybir.AluOpType.mult,
                    ).then_inc(sAh, 1)
                for b in range(NBLK):
                    vector.wait_ge(sRd[b], 16)
                    at = accT[b % 2]
                    ev = bass.AP(at.tensor, at.offset,
                                 [list(at.ap[0]), [2 * D, OCOLS], [1, D]])
                    od = bass.AP(at.tensor, at.offset + D,
                                 [list(at.ap[0]), [2 * D, OCOLS], [1, D]])
                    if b == 0:
                        vector.tensor_tensor(
                            accm[:], ev, od, op=mybir.AluOpType.add
                        ).then_inc(sMg, 1)
                    else:
                        vector.tensor_tensor(
                            accm[:], accm[:], ev, op=mybir.AluOpType.add
                        ).then_inc(sMg, 1)
                        vector.tensor_tensor(
                            accm[:], accm[:], od, op=mybir.AluOpType.add
                        ).then_inc(sMg, 1)
                dvD = bcast(dinvDs[:], D)
                vector.tensor_tensor(
                    accm[:], accm[:], dvD, op=mybir.AluOpType.mult
                ).then_inc(sMg, 1)
                apb = bBCs[:]
                bb = bass.AP(apb.tensor, apb.offset,
                             [list(apb.ap[0]), [0, OCOLS],
                              list(apb.ap[1])])
                vector.tensor_tensor(
                    outT, accm[:], bb, op=mybir.AluOpType.add
                ).then_inc(sMg, 1)

            @block.scalar
            def _(scalar):
                scalar.wait_ge(sDin, 1)
                scalar.activation(dinvTs[:], dinvTs[:],
                                  mybir.ActivationFunctionType.Sqrt)
                scalar.activation(dinvDs[:], dinvDs[:],
                                  mybir.ActivationFunctionType.Sqrt
                                  ).then_inc(sDin, 1)

            @block.gpsimd
            def _(gpsimd: bass.BassGpSimd):
                gpsimd.load_library(mlp)
                gpsimd.wait_ge(sLD, 80)
                for i, (a, w, ioff, treq) in enumerate(sched):
                    b = i % NBLK
                    gpsimd.wait_ge(sAh, treq)
                    if i >= NBLK:
                        gpsimd.wait_ge(sDma[b], (i // NBLK) * 16)
                    out_ap = bass.AP(t_acc[b], 0,
                                     [[128, ACCROWS], [1, D]])
                    gpsimd.dma_scatter_add(
                        out_ap,
                        hp[:, a:a + w, :],
                        scr[:, ioff:ioff + 8 * w],
                        128 * w, 128 * w, D, elem_step=128,
                    ).then_inc(sDma[b], 16)

        nc.compile()
    return nc


def kernel(x, edge_index, W, b):
    x = np.asarray(x, dtype=np.float32)
    edge_index = np.asarray(edge_index)
    W = np.asarray(W, dtype=np.float32)
    b = np.asarray(b, dtype=np.float32)

    src = np.asarray(edge_index[0], dtype=np.int64)
    dst = np.asarray(edge_index[1], dtype=np.int64)

    deg_glob = (np.bincount(dst, minlength=N) + 1.0).astype(np.float32)

    cores = []
    degs = []
    for c in range(NCORES):
        m = (dst >= c * SHARD) & (dst < (c + 1) * SHARD)
        sg = np.concatenate([src[m], np.arange(c * SHARD, (c + 1) * SHARD)])
        dl = np.concatenate([dst[m] - c * SHARD, np.arange(SHARD)])
        cores.append((sg, dl))
        degs.append(np.bincount(sg, minlength=N))
    maxdeg = max(int(d.max()) for d in degs)
    K = np.array([max(int((d >= r).sum()) for d in degs)
                  for r in range(1, maxdeg + 1)])

    calls, KCOLS = _build_geometry(K)
    solved = [_solve_core(*cores[c], K, calls, seed=c * 7 + 1)
              for c in range(NCORES)]

    treqs = [-(-(a + w) // 8) for (a, w, lev) in calls]
    cum = np.cumsum([0] + [8 * w for (a, w, lev) in calls])
    order_c = sorted(range(len(calls)), key=lambda i: treqs[i])
    sched = [(calls[i][0], calls[i][1], int(cum[i]), treqs[i])
             for i in order_c]
    IDXC = max(int(cum[-1]), 2 * ACOLS * D)

    WT16 = np.ascontiguousarray(W.T).astype(np.float16)
    bBC = np.ascontiguousarray(np.broadcast_to(b, (128, D))
                               ).astype(np.float32)

    in_maps = []
    for c in range(NCORES):
        order, per_call = solved[c]
        sel = order[:KCOLS * 128]
        xT = np.ascontiguousarray(x[sel].astype(np.float16).T)
        degT = np.ascontiguousarray(
            deg_glob[sel].reshape(KCOLS, 128).T).astype(np.float32)
        degD_flat = np.ones(OUTROWS, np.float32)
        degD_flat[:SHARD] = deg_glob[c * SHARD:(c + 1) * SHARD]
        degD = np.ascontiguousarray(degD_flat.reshape(128, OCOLS))
        idx_tbl = np.zeros((128, IDXC), np.int16)
        for i, arr in enumerate(per_call):
            idx_tbl[:, int(cum[i]):int(cum[i + 1])] = _wrap16(arr)
        in_maps.append({
            "xT": xT, "WT": WT16, "degT": degT, "degD": degD,
            "bBC": bBC, "idx": idx_tbl,
        })

    nc = _build_program(KCOLS, IDXC, sched)
    global LAST_NC
    LAST_NC = nc
    res = bass_utils.run_bass_kernel_spmd(nc, in_maps,
                                          core_ids=list(range(NCORES)))
    out = np.empty((N, D), np.float32)
    for c in range(NCORES):
        o16 = np.asarray(res.results[c]["out_s"])
        out[c * SHARD:(c + 1) * SHARD] = o16[:SHARD].astype(np.float32)
    return out


# revision 3
# speedup vs baseline: 1.0247x; 1.0247x over previous
"""GCNConv (PyG semantics) on 8 Trainium2 NeuronCores — scatter-add design.

out = D^-1/2 (A+I) D^-1/2 (x @ W.T) + b, dst-sharded across 8 cores.

Host: per core, sort nodes by core-local out-degree (desc). Level r covers
the sorted prefix of nodes with deg >= r; levels split into calls of <= 48
columns (6144 positions). Each node's edges map bijectively onto its levels'
calls (+ level-free overflow calls over the low-degree band). Per call,
every assigned accumulator row (dst*2 + replica) is unique (host solver);
unfilled positions scatter to trash rows.

Device, per core:
  Phase A: h' = (x @ W.T) * dinv[src] -> SBUF-resident fp16 table
           [128, KCOLS, 64] (sorted position j -> (j%128, j//128)).
  Phase B: per call (ordered by required h' column): dma_scatter_add of its
           position range into acc block (chain_index % 3). The 3 blocks
           are separate pre-zeroed DRAM tensors; call i waits on call
           i-3's DMA completion, so concurrent transfers always target
           distinct blocks and the CCE read-modify-write never races on a
           row (within-call rows are unique by construction).
  Phase C: read blocks back (only the used 64-elem half of each 128-elem
           row), merge the two replica sub-rows, scale by dinv[dst], add
           bias, write out shard (fp16; host casts to fp32).
"""

import numpy as np
from collections import defaultdict
from contextlib import ExitStack

import concourse.bacc as bacc
import concourse.bass as bass
import concourse.mybir as mybir
from concourse import bass_utils
from concourse.library_config import mlp

D = 64
N = 100000
NCORES = 8
SHARD = N // NCORES          # 12500
OUTROWS = 12544              # 128 * 98
OCOLS = OUTROWS // 128       # 98
ACCROWS = 2 * SHARD + 88     # 25088 rows per block; [25000,25088) = trash
TRASH0 = 2 * SHARD
ACOLS = ACCROWS // 128       # 196
NBLK = 3
CALLCOLS = 48

LAST_NC = None


def _wrap16(idx_flat):
    # scatter-add desc-gen runs on Q7 cores 0-1 only; they read idx from
    # partitions 0-31 (16-wrap x 2 replicas) — verified on HW
    n = idx_flat.shape[0]
    out = idx_flat.reshape(n // 16, 16).T.astype(np.int16)
    return np.tile(out, (2, 1))


def _build_geometry(K):
    cols = [-(-int(k) // 128) for k in K]
    calls = []
    for r in range(len(K)):
        for a in range(0, cols[r], CALLCOLS):
            calls.append((a, min(CALLCOLS, cols[r] - a), r))
    band_lo = cols[1] if len(K) > 1 else 0
    for a in range(band_lo, cols[0], CALLCOLS):
        calls.append((a, min(CALLCOLS, cols[0] - a), -1))
    kcols = -(-cols[0] // 8) * 8
    return calls, kcols


def _solve_core(src_g, dst_l, K, calls, seed):
    """Assign each edge to (call, acc_row) with rows unique per call."""
    rng = np.random.default_rng(seed)
    E = src_g.shape[0]
    deg = np.bincount(src_g, minlength=N)
    order = np.argsort(-deg, kind="stable")
    pos_of = np.empty(N, np.int64)
    pos_of[order] = np.arange(N)

    prim = {}
    ovf_of_col = defaultdict(list)
    for i, (a, w, lev) in enumerate(calls):
        for cc in range(a, a + w):
            if lev < 0:
                ovf_of_col[cc].append(i)
            else:
                prim[(lev, cc)] = i

    p = pos_of[src_g]
    o = np.lexsort((rng.random(E), p))
    p_s = p[o]
    dst_s = dst_l[o].astype(np.int64)
    new = np.ones(E, bool)
    new[1:] = p_s[1:] != p_s[:-1]
    starts = np.nonzero(new)[0]
    grp_id = np.cumsum(new) - 1
    glen = np.diff(np.append(starts, E))
    pos_g = p_s[starts].copy()          # group -> position (mutable)
    grp_at_pos = {int(pos_g[g]): g for g in range(len(starts))}

    cid = np.full(E, -1, np.int64)
    row = np.full(E, -1, np.int64)
    slot_of = {}     # call*32768 + row -> edge
    callee = {}      # grp*256 + call -> edge

    def cand_calls(e):
        g = grp_id[e]
        c = int(pos_g[g]) // 128
        cl = [prim[(r, c)] for r in range(int(glen[g]))]
        return cl + ovf_of_col.get(c, [])

    def do_place(e, c2, r2):
        cid[e] = c2
        row[e] = r2
        slot_of[c2 * 32768 + r2] = e
        callee[int(grp_id[e]) * 256 + c2] = e

    def un_place(e):
        slot_of.pop(int(cid[e]) * 32768 + int(row[e]), None)
        callee.pop(int(grp_id[e]) * 256 + int(cid[e]), None)
        cid[e] = -1
        row[e] = -1

    def try_place(e, depth, vis):
        if e in vis:
            return False
        vis.add(e)
        g = int(grp_id[e])
        d2 = int(dst_s[e]) * 2
        cl = cand_calls(e)
        rng.shuffle(cl)
        for c2 in cl:
            if g * 256 + c2 in callee:
                continue
            for r2 in (d2, d2 + 1):
                if c2 * 32768 + r2 not in slot_of:
                    do_place(e, c2, r2)
                    return True
        if depth == 0:
            return False
        for c2 in cl:
            sib = callee.get(g * 256 + c2)
            if sib is not None:
                if sib == e:
                    continue
                sc, sr = int(cid[sib]), int(row[sib])
                free_r = None
                for r2 in (d2, d2 + 1):
                    occ = slot_of.get(c2 * 32768 + r2)
                    if occ is None or occ == sib:
                        free_r = r2
                        break
                if free_r is None:
                    continue
                un_place(sib)
                if c2 * 32768 + free_r in slot_of:
                    do_place(sib, sc, sr)
                    continue
                do_place(e, c2, free_r)
                if try_place(sib, depth - 1, vis):
                    return True
                un_place(e)
                do_place(sib, sc, sr)
                continue
            for r2 in (d2, d2 + 1):
                b = slot_of.get(c2 * 32768 + r2)
                if b is None:
                    do_place(e, c2, r2)
                    return True
                bc, br = int(cid[b]), int(row[b])
                un_place(b)
                do_place(e, c2, r2)
                if try_place(b, depth - 1, vis):
                    return True
                un_place(e)
                do_place(b, bc, br)
        return False

    order_e = np.arange(E)
    rng.shuffle(order_e)
    for e in order_e:
        try_place(int(e), 0, set())
    for depth in (1, 2, 3, 4):
        left = np.nonzero(cid < 0)[0]
        if left.size == 0:
            break
        for e in left:
            try_place(int(e), depth, set())

    # node-swap escape for the stubborn tail
    left = np.nonzero(cid < 0)[0]
    for e in left:
        e = int(e)
        if cid[e] >= 0:
            continue
        g = int(grp_id[e])
        dg = int(glen[g])
        lo = int(K[dg]) if dg < len(K) else 0
        hi = int(K[dg - 1])
        done = False
        for _ in range(500):
            v_pos = int(rng.integers(lo, hi))
            vg = grp_at_pos.get(v_pos)
            if vg is None or vg == g or int(glen[vg]) != dg:
                continue
            if v_pos // 128 == int(pos_g[g]) // 128:
                continue
            ues = list(range(int(starts[g]), int(starts[g]) + dg))
            ves = list(range(int(starts[vg]), int(starts[vg]) + dg))
            saved = [(int(cid[x]), int(row[x])) for x in ues + ves]
            for x in ues + ves:
                if cid[x] >= 0:
                    un_place(x)
            u_pos = int(pos_g[g])
            pos_g[g], pos_g[vg] = v_pos, u_pos
            grp_at_pos[v_pos], grp_at_pos[u_pos] = g, vg
            ok = True
            for x in ues + ves:
                if not try_place(x, 2, set()):
                    ok = False
                    break
            if ok:
                u_node = order[u_pos]
                v_node = order[v_pos]
                order[u_pos], order[v_pos] = v_node, u_node
                done = True
                break
            for x in ues + ves:
                if cid[x] >= 0:
                    un_place(x)
            pos_g[g], pos_g[vg] = u_pos, v_pos
            grp_at_pos[u_pos], grp_at_pos[v_pos] = g, vg
            for x, (sc, sr) in zip(ues + ves, saved):
                if sc >= 0:
                    do_place(x, sc, sr)
        if not done and cid[e] < 0:
            raise RuntimeError("node-swap failed for an edge")

    if int((cid < 0).sum()):
        raise RuntimeError("solver left edges unplaced")
    keys = cid * 32768 + row
    if len(np.unique(keys)) != E:
        raise RuntimeError("solver produced duplicate (call,row)")

    per_call = []
    for i, (a, w, lev) in enumerate(calls):
        n = 128 * w
        per_call.append(TRASH0 + (np.arange(n) % (ACCROWS - TRASH0)))
    # scatter positions: edge at table position pos_g[grp] + rank-in-group?
    # positions of edges within a group all equal the node position; the
    # edge's slot inside its call is node position - call base.
    for e in range(E):
        c2 = int(cid[e])
        a = calls[c2][0]
        pos = int(pos_g[int(grp_id[e])])
        per_call[c2][pos - a * 128] = int(row[e])
    return order, per_call


def _build_program(KCOLS, IDXC, sched):
    dt = mybir.dt
    AIT = KCOLS // 8
    NCALLS = len(sched)

    nc = bacc.Bacc("TRN2", target_bir_lowering=False, debug=False,
                   num_devices=NCORES)
    t_xT = nc.dram_tensor("xT", [D, KCOLS * 128], dt.float16,
                          kind="ExternalInput")
    t_WT = nc.dram_tensor("WT", [D, D], dt.float16, kind="ExternalInput")
    t_degT = nc.dram_tensor("degT", [128, KCOLS], dt.float32,
                            kind="ExternalInput")
    t_degD = nc.dram_tensor("degD", [128, OCOLS], dt.float32,
                            kind="ExternalInput")
    t_bBC = nc.dram_tensor("bBC", [128, D], dt.float16,
                           kind="ExternalInput")
    t_idx = nc.dram_tensor("idx", [32, IDXC], dt.int16,
                           kind="ExternalInput")
    t_acc = [nc.dram_tensor(f"acc{b}", [ACCROWS, 128], dt.float16,
                            kind="ExternalOutput") for b in range(NBLK)]
    t_out = nc.dram_tensor("out_s", [OUTROWS, D], dt.float16,
                           kind="ExternalOutput")

    cnt_blk = [0] * NBLK
    for i in range(NCALLS):
        cnt_blk[i % NBLK] += 1

    with ExitStack() as ctx:
        e = ctx.enter_context
        hp = e(nc.sbuf_tensor("hp", [128, KCOLS, D], dt.float16))
        scr = e(nc.sbuf_tensor("scr", [128, IDXC], dt.int16))
        xb = [e(nc.sbuf_tensor(f"xb{i}", [D, 1024], dt.float16))
              for i in range(4)]
        WTs = e(nc.sbuf_tensor("WTs", [D, D], dt.float16))
        degTs = e(nc.sbuf_tensor("degTs", [128, KCOLS], dt.float32))
        dinvTs = e(nc.sbuf_tensor("dinvTs", [128, KCOLS], dt.float32))
        degDs = e(nc.sbuf_tensor("degDs", [128, OCOLS], dt.float32))
        dinvDs = e(nc.sbuf_tensor("dinvDs", [128, OCOLS], dt.float32))
        bBCs = e(nc.sbuf_tensor("bBCs", [128, D], dt.float16))
        accm = e(nc.sbuf_tensor("accm", [128, OCOLS, D], dt.float16))
        dinvD16 = e(nc.sbuf_tensor("dinvD16", [128, OCOLS], dt.float16))
        psum = [e(nc.psum_tensor(f"ps{i}", [128, 512], dt.float32))
                for i in range(8)]

        sLD = e(nc.semaphore("sLD"))
        sDin = e(nc.semaphore("sDin"))
        sAx = [e(nc.semaphore(f"sAx{i}")) for i in range(4)]
        sAmm = e(nc.semaphore("sAmm"))
        sAh = e(nc.semaphore("sAh"))
        sDma = [e(nc.semaphore(f"sDma{b}")) for b in range(NBLK)]
        sRd = [e(nc.semaphore(f"sRd{b}")) for b in range(NBLK)]
        sMg = e(nc.semaphore("sMg"))
        sFin = e(nc.semaphore("sFin"))

        # aliased views over scr (idx dead by the time these are used)
        accT = [scr[:, b * ACOLS * D:(b + 1) * ACOLS * D]
                .bitcast(dt.float16).rearrange("p (c d) -> p c d", d=D)
                for b in range(2)]
        # outT aliases accT[0]'s bytes: block-2 data there is fully consumed
        # by the (sequential) vector merges before the bias op writes outT
        outT = scr[:, 0:OCOLS * D] \
            .bitcast(dt.float16).rearrange("p (c d) -> p c d", d=D)

        def bcast(ap, reps):
            return bass.AP(ap.tensor, ap.offset, list(ap.ap) + [[0, reps]])

        with nc.Block() as block:

            @block.sync
            def _(sync: bass.BassEngine):
                sync.dma_start(WTs[:], t_WT[:]).then_inc(sLD, 16)
                sync.dma_start(degTs[:], t_degT[:]).then_inc(sLD, 16)
                sync.dma_start(degDs[:], t_degD[:]).then_inc(sLD, 16)
                sync.dma_start(bBCs[:], t_bBC[:]).then_inc(sLD, 16)
                sync.dma_start(scr[0:32, :], t_idx[:]
                               ).then_inc(sLD, 16)
                for it in range(AIT):
                    if it >= 4:
                        sync.wait_ge(sAmm, it - 3)
                    sync.dma_start(
                        xb[it % 4][:],
                        t_xT[:, it * 1024:(it + 1) * 1024],
                    ).then_inc(sAx[it % 4], 16)
                for b in range(NBLK):
                    sync.wait_ge(sDma[b], cnt_blk[b] * 16)
                for b in range(NBLK):
                    if b >= 2:
                        sync.wait_ge(sMg, 1)
                    src = bass.AP(t_acc[b], 0,
                                  [[ACOLS * 128, 128], [128, ACOLS],
                                   [1, D]])
                    sync.dma_start(accT[b % 2], src).then_inc(sRd[b], 16)
                sync.wait_ge(sMg, 7)
                out3 = bass.AP(t_out, 0,
                               [[OCOLS * D, 128], [D, OCOLS], [1, D]])
                sync.dma_start(out3, outT).then_inc(sFin, 16)
                sync.wait_ge(sFin, 16)

            @block.tensor
            def _(tensor):
                tensor.wait_ge(sLD, 80)  # DMA completions may reorder
                for it in range(AIT):
                    tensor.wait_ge(sAx[it % 4], (it // 4 + 1) * 16)
                    if it >= 8:
                        tensor.wait_ge(sAh, it - 7)
                    for j in range(8):
                        ins = tensor.matmul(
                            psum[it % 8][:, j * D:(j + 1) * D],
                            xb[it % 4][:, j * 128:(j + 1) * 128],
                            WTs[:],
                            start=True, stop=True,
                        )
                    ins.then_inc(sAmm, 1)

            @block.vector
            def _(vector):
                vector.wait_ge(sLD, 80)  # DMA completions may reorder
                vector.reciprocal(dinvTs[:], degTs[:])
                vector.reciprocal(dinvDs[:], degDs[:]).then_inc(sDin, 1)
                vector.wait_ge(sDin, 2)
                for it in range(AIT):
                    vector.wait_ge(sAmm, it + 1)
                    ps3 = psum[it % 8][:].rearrange("p (c d) -> p c d", d=D)
                    dv = bcast(dinvTs[:, it * 8:(it + 1) * 8], D)
                    vector.tensor_tensor(
                        hp[:, it * 8:(it + 1) * 8, :], ps3, dv,
                        op=mybir.AluOpType.mult,
                    ).then_inc(sAh, 1)
                for b in range(NBLK):
                    vector.wait_ge(sRd[b], 16)
                    at = accT[b % 2]
                    ev = bass.AP(at.tensor, at.offset,
                                 [list(at.ap[0]), [2 * D, OCOLS], [1, D]])
                    od = bass.AP(at.tensor, at.offset + D,
                                 [list(at.ap[0]), [2 * D, OCOLS], [1, D]])
                    if b == 0:
                        vector.tensor_tensor(
                            accm[:], ev, od, op=mybir.AluOpType.add
                        ).then_inc(sMg, 1)
                    else:
                        vector.tensor_tensor(
                            accm[:], accm[:], ev, op=mybir.AluOpType.add
                        ).then_inc(sMg, 1)
                        vector.tensor_tensor(
                            accm[:], accm[:], od, op=mybir.AluOpType.add
                        ).then_inc(sMg, 1)
                dvD = bcast(dinvD16[:], D)
                vector.tensor_tensor(
                    accm[:], accm[:], dvD, op=mybir.AluOpType.mult
                ).then_inc(sMg, 1)
                apb = bBCs[:]
                bb = bass.AP(apb.tensor, apb.offset,
                             [list(apb.ap[0]), [0, OCOLS],
                              list(apb.ap[1])])
                vector.tensor_tensor(
                    outT, accm[:], bb, op=mybir.AluOpType.add
                ).then_inc(sMg, 1)

            @block.scalar
            def _(scalar):
                scalar.wait_ge(sDin, 1)
                scalar.activation(dinvTs[:], dinvTs[:],
                                  mybir.ActivationFunctionType.Sqrt)
                scalar.activation(dinvDs[:], dinvDs[:],
                                  mybir.ActivationFunctionType.Sqrt)
                scalar.activation(dinvD16[:], dinvDs[:],
                                  mybir.ActivationFunctionType.Copy
                                  ).then_inc(sDin, 1)

            @block.gpsimd
            def _(gpsimd: bass.BassGpSimd):
                gpsimd.load_library(mlp)
                gpsimd.wait_ge(sLD, 80)
                for i, (a, w, ioff, treq) in enumerate(sched):
                    b = i % NBLK
                    gpsimd.wait_ge(sAh, treq)
                    if i >= NBLK:
                        gpsimd.wait_ge(sDma[b], (i // NBLK) * 16)
                    out_ap = bass.AP(t_acc[b], 0,
                                     [[128, ACCROWS], [1, D]])
                    gpsimd.dma_scatter_add(
                        out_ap,
                        hp[:, a:a + w, :],
                        scr[:, ioff:ioff + 8 * w],
                        128 * w, 128 * w, D, elem_step=128,
                    ).then_inc(sDma[b], 16)

        nc.compile()
    return nc


def kernel(x, edge_index, W, b):
    x = np.asarray(x, dtype=np.float32)
    edge_index = np.asarray(edge_index)
    W = np.asarray(W, dtype=np.float32)
    b = np.asarray(b, dtype=np.float32)

    src = np.asarray(edge_index[0], dtype=np.int64)
    dst = np.asarray(edge_index[1], dtype=np.int64)

    deg_glob = (np.bincount(dst, minlength=N) + 1.0).astype(np.float32)

    cores = []
    degs = []
    for c in range(NCORES):
        m = (dst >= c * SHARD) & (dst < (c + 1) * SHARD)
        sg = np.concatenate([src[m], np.arange(c * SHARD, (c + 1) * SHARD)])
        dl = np.concatenate([dst[m] - c * SHARD, np.arange(SHARD)])
        cores.append((sg, dl))
        degs.append(np.bincount(sg, minlength=N))
    maxdeg = max(int(d.max()) for d in degs)
    K = np.array([max(int((d >= r).sum()) for d in degs)
                  for r in range(1, maxdeg + 1)])

    calls, KCOLS = _build_geometry(K)
    solved = [_solve_core(*cores[c], K, calls, seed=c * 7 + 1)
              for c in range(NCORES)]

    treqs = [-(-(a + w) // 8) for (a, w, lev) in calls]
    cum = np.cumsum([0] + [8 * w for (a, w, lev) in calls])
    order_c = sorted(range(len(calls)), key=lambda i: treqs[i])
    sched = [(calls[i][0], calls[i][1], int(cum[i]), treqs[i])
             for i in order_c]
    IDXC = max(int(cum[-1]), 2 * ACOLS * D)

    WT16 = np.ascontiguousarray(W.T).astype(np.float16)
    bBC = np.ascontiguousarray(np.broadcast_to(b, (128, D))
                               ).astype(np.float16)

    in_maps = []
    for c in range(NCORES):
        order, per_call = solved[c]
        sel = order[:KCOLS * 128]
        xT = np.ascontiguousarray(x[sel].astype(np.float16).T)
        degT = np.ascontiguousarray(
            deg_glob[sel].reshape(KCOLS, 128).T).astype(np.float32)
        degD_flat = np.ones(OUTROWS, np.float32)
        degD_flat[:SHARD] = deg_glob[c * SHARD:(c + 1) * SHARD]
        degD = np.ascontiguousarray(degD_flat.reshape(128, OCOLS))
        idx_tbl = np.zeros((32, IDXC), np.int16)
        for i, arr in enumerate(per_call):
            idx_tbl[:, int(cum[i]):int(cum[i + 1])] = _wrap16(arr)
        in_maps.append({
            "xT": xT, "WT": WT16, "degT": degT, "degD": degD,
            "bBC": bBC, "idx": idx_tbl,
        })

    nc = _build_program(KCOLS, IDXC, sched)
    global LAST_NC
    LAST_NC = nc
    res = bass_utils.run_bass_kernel_spmd(nc, in_maps,
                                          core_ids=list(range(NCORES)))
    out = np.empty((N, D), np.float32)
    for c in range(NCORES):
        o16 = np.asarray(res.results[c]["out_s"])
        out[c * SHARD:(c + 1) * SHARD] = o16[:SHARD].astype(np.float32)
    return out


# revision 4
# speedup vs baseline: 1.0248x; 1.0001x over previous
"""GCNConv (PyG semantics) on 8 Trainium2 NeuronCores — scatter-add design.

out = D^-1/2 (A+I) D^-1/2 (x @ W.T) + b, dst-sharded across 8 cores.

Host: per core, sort nodes by core-local out-degree (desc). Level r covers
the sorted prefix of nodes with deg >= r; levels split into calls of <= 48
columns (6144 positions). Each node's edges map bijectively onto its levels'
calls (+ level-free overflow calls over the low-degree band). Per call,
every assigned accumulator row (dst*2 + replica) is unique (host solver);
unfilled positions scatter to trash rows.

Device, per core:
  Phase A: h' = (x @ W.T) * dinv[src] -> SBUF-resident fp16 table
           [128, KCOLS, 64] (sorted position j -> (j%128, j//128)).
  Phase B: per call (ordered by required h' column): dma_scatter_add of its
           position range into acc block (chain_index % 3). The 3 blocks
           are separate pre-zeroed DRAM tensors; call i waits on call
           i-3's DMA completion, so concurrent transfers always target
           distinct blocks and the CCE read-modify-write never races on a
           row (within-call rows are unique by construction).
  Phase C: read blocks back (only the used 64-elem half of each 128-elem
           row), merge the two replica sub-rows, scale by dinv[dst], add
           bias, write out shard (fp16; host casts to fp32).
"""

import numpy as np
from collections import defaultdict
from contextlib import ExitStack

import concourse.bacc as bacc
import concourse.bass as bass
import concourse.mybir as mybir
from concourse import bass_utils
from concourse.library_config import mlp

D = 64
N = 100000
NCORES = 8
SHARD = N // NCORES          # 12500
OUTROWS = 12544              # 128 * 98
OCOLS = OUTROWS // 128       # 98
ACCROWS = 2 * SHARD + 88     # 25088 rows per block; [25000,25088) = trash
TRASH0 = 2 * SHARD
ACOLS = ACCROWS // 128       # 196
NBLK = 3
CALLCOLS = 48

LAST_NC = None


def _wrap16(idx_flat):
    # scatter-add desc-gen runs on Q7 cores 0-1 only; they read idx from
    # partitions 0-31 (16-wrap x 2 replicas) — verified on HW
    n = idx_flat.shape[0]
    out = idx_flat.reshape(n // 16, 16).T.astype(np.int16)
    return np.tile(out, (2, 1))


def _build_geometry(K):
    cols = [-(-int(k) // 128) for k in K]
    calls = []
    for r in range(len(K)):
        for a in range(0, cols[r], CALLCOLS):
            calls.append((a, min(CALLCOLS, cols[r] - a), r))
    band_lo = cols[1] if len(K) > 1 else 0
    for a in range(band_lo, cols[0], CALLCOLS):
        calls.append((a, min(CALLCOLS, cols[0] - a), -1))
    kcols = -(-cols[0] // 8) * 8
    return calls, kcols


def _solve_core(src_g, dst_l, K, calls, seed):
    """Assign each edge to (call, acc_row) with rows unique per call."""
    rng = np.random.default_rng(seed)
    E = src_g.shape[0]
    deg = np.bincount(src_g, minlength=N)
    order = np.argsort(-deg, kind="stable")
    pos_of = np.empty(N, np.int64)
    pos_of[order] = np.arange(N)

    prim = {}
    ovf_of_col = defaultdict(list)
    for i, (a, w, lev) in enumerate(calls):
        for cc in range(a, a + w):
            if lev < 0:
                ovf_of_col[cc].append(i)
            else:
                prim[(lev, cc)] = i

    p = pos_of[src_g]
    o = np.lexsort((rng.random(E), p))
    p_s = p[o]
    dst_s = dst_l[o].astype(np.int64)
    new = np.ones(E, bool)
    new[1:] = p_s[1:] != p_s[:-1]
    starts = np.nonzero(new)[0]
    grp_id = np.cumsum(new) - 1
    glen = np.diff(np.append(starts, E))
    pos_g = p_s[starts].copy()          # group -> position (mutable)
    grp_at_pos = {int(pos_g[g]): g for g in range(len(starts))}

    cid = np.full(E, -1, np.int64)
    row = np.full(E, -1, np.int64)
    slot_of = {}     # call*32768 + row -> edge
    callee = {}      # grp*256 + call -> edge

    def cand_calls(e):
        g = grp_id[e]
        c = int(pos_g[g]) // 128
        cl = [prim[(r, c)] for r in range(int(glen[g]))]
        return cl + ovf_of_col.get(c, [])

    def do_place(e, c2, r2):
        cid[e] = c2
        row[e] = r2
        slot_of[c2 * 32768 + r2] = e
        callee[int(grp_id[e]) * 256 + c2] = e

    def un_place(e):
        slot_of.pop(int(cid[e]) * 32768 + int(row[e]), None)
        callee.pop(int(grp_id[e]) * 256 + int(cid[e]), None)
        cid[e] = -1
        row[e] = -1

    def try_place(e, depth, vis):
        if e in vis:
            return False
        vis.add(e)
        g = int(grp_id[e])
        d2 = int(dst_s[e]) * 2
        cl = cand_calls(e)
        rng.shuffle(cl)
        for c2 in cl:
            if g * 256 + c2 in callee:
                continue
            for r2 in (d2, d2 + 1):
                if c2 * 32768 + r2 not in slot_of:
                    do_place(e, c2, r2)
                    return True
        if depth == 0:
            return False
        for c2 in cl:
            sib = callee.get(g * 256 + c2)
            if sib is not None:
                if sib == e:
                    continue
                sc, sr = int(cid[sib]), int(row[sib])
                free_r = None
                for r2 in (d2, d2 + 1):
                    occ = slot_of.get(c2 * 32768 + r2)
                    if occ is None or occ == sib:
                        free_r = r2
                        break
                if free_r is None:
                    continue
                un_place(sib)
                if c2 * 32768 + free_r in slot_of:
                    do_place(sib, sc, sr)
                    continue
                do_place(e, c2, free_r)
                if try_place(sib, depth - 1, vis):
                    return True
                un_place(e)
                do_place(sib, sc, sr)
                continue
            for r2 in (d2, d2 + 1):
                b = slot_of.get(c2 * 32768 + r2)
                if b is None:
                    do_place(e, c2, r2)
                    return True
                bc, br = int(cid[b]), int(row[b])
                un_place(b)
                do_place(e, c2, r2)
                if try_place(b, depth - 1, vis):
                    return True
                un_place(e)
                do_place(b, bc, br)
        return False

    order_e = np.arange(E)
    rng.shuffle(order_e)
    for e in order_e:
        try_place(int(e), 0, set())
    for depth in (1, 2, 3, 4):
        left = np.nonzero(cid < 0)[0]
        if left.size == 0:
            break
        for e in left:
            try_place(int(e), depth, set())

    # node-swap escape for the stubborn tail
    left = np.nonzero(cid < 0)[0]
    for e in left:
        e = int(e)
        if cid[e] >= 0:
            continue
        g = int(grp_id[e])
        dg = int(glen[g])
        lo = int(K[dg]) if dg < len(K) else 0
        hi = int(K[dg - 1])
        done = False
        for _ in range(500):
            v_pos = int(rng.integers(lo, hi))
            vg = grp_at_pos.get(v_pos)
            if vg is None or vg == g or int(glen[vg]) != dg:
                continue
            if v_pos // 128 == int(pos_g[g]) // 128:
                continue
            ues = list(range(int(starts[g]), int(starts[g]) + dg))
            ves = list(range(int(starts[vg]), int(starts[vg]) + dg))
            saved = [(int(cid[x]), int(row[x])) for x in ues + ves]
            for x in ues + ves:
                if cid[x] >= 0:
                    un_place(x)
            u_pos = int(pos_g[g])
            pos_g[g], pos_g[vg] = v_pos, u_pos
            grp_at_pos[v_pos], grp_at_pos[u_pos] = g, vg
            ok = True
            for x in ues + ves:
                if not try_place(x, 2, set()):
                    ok = False
                    break
            if ok:
                u_node = order[u_pos]
                v_node = order[v_pos]
                order[u_pos], order[v_pos] = v_node, u_node
                done = True
                break
            for x in ues + ves:
                if cid[x] >= 0:
                    un_place(x)
            pos_g[g], pos_g[vg] = u_pos, v_pos
            grp_at_pos[u_pos], grp_at_pos[v_pos] = g, vg
            for x, (sc, sr) in zip(ues + ves, saved):
                if sc >= 0:
                    do_place(x, sc, sr)
        if not done and cid[e] < 0:
            raise RuntimeError("node-swap failed for an edge")

    if int((cid < 0).sum()):
        raise RuntimeError("solver left edges unplaced")
    keys = cid * 32768 + row
    if len(np.unique(keys)) != E:
        raise RuntimeError("solver produced duplicate (call,row)")

    per_call = []
    for i, (a, w, lev) in enumerate(calls):
        n = 128 * w
        per_call.append(TRASH0 + (np.arange(n) % (ACCROWS - TRASH0)))
    # scatter positions: edge at table position pos_g[grp] + rank-in-group?
    # positions of edges within a group all equal the node position; the
    # edge's slot inside its call is node position - call base.
    for e in range(E):
        c2 = int(cid[e])
        a = calls[c2][0]
        pos = int(pos_g[int(grp_id[e])])
        per_call[c2][pos - a * 128] = int(row[e])
    return order, per_call


def _build_program(KCOLS, IDXC, sched):
    dt = mybir.dt
    AIT = KCOLS // 8
    NCALLS = len(sched)

    nc = bacc.Bacc("TRN2", target_bir_lowering=False, debug=False,
                   num_devices=NCORES)
    t_xT = nc.dram_tensor("xT", [D, KCOLS * 128], dt.float16,
                          kind="ExternalInput")
    t_WT = nc.dram_tensor("WT", [D, D], dt.float16, kind="ExternalInput")
    t_degT = nc.dram_tensor("degT", [128, KCOLS], dt.float32,
                            kind="ExternalInput")
    t_degD = nc.dram_tensor("degD", [128, OCOLS], dt.float32,
                            kind="ExternalInput")
    t_bBC = nc.dram_tensor("bBC", [128, D], dt.float16,
                           kind="ExternalInput")
    t_idx = nc.dram_tensor("idx", [32, IDXC], dt.int16,
                           kind="ExternalInput")
    t_acc = [nc.dram_tensor(f"acc{b}", [ACCROWS, 128], dt.float16,
                            kind="ExternalOutput") for b in range(NBLK)]
    t_out = nc.dram_tensor("out_s", [OUTROWS, D], dt.float16,
                           kind="ExternalOutput")

    cnt_blk = [0] * NBLK
    for i in range(NCALLS):
        cnt_blk[i % NBLK] += 1

    with ExitStack() as ctx:
        e = ctx.enter_context
        hp = e(nc.sbuf_tensor("hp", [128, KCOLS, D], dt.float16))
        scr = e(nc.sbuf_tensor("scr", [128, IDXC], dt.int16))
        xb = [e(nc.sbuf_tensor(f"xb{i}", [D, 1024], dt.float16))
              for i in range(4)]
        WTs = e(nc.sbuf_tensor("WTs", [D, D], dt.float16))
        degTs = e(nc.sbuf_tensor("degTs", [128, KCOLS], dt.float32))
        dinvTs = e(nc.sbuf_tensor("dinvTs", [128, KCOLS], dt.float32))
        degDs = e(nc.sbuf_tensor("degDs", [128, OCOLS], dt.float32))
        dinvDs = e(nc.sbuf_tensor("dinvDs", [128, OCOLS], dt.float32))
        bBCs = e(nc.sbuf_tensor("bBCs", [128, D], dt.float16))
        accm = e(nc.sbuf_tensor("accm", [128, OCOLS, D], dt.float16))
        dinvD16 = e(nc.sbuf_tensor("dinvD16", [128, OCOLS], dt.float16))
        psum = [e(nc.psum_tensor(f"ps{i}", [128, 512], dt.float32))
                for i in range(8)]

        sLD = e(nc.semaphore("sLD"))
        sDin = e(nc.semaphore("sDin"))
        sAx = [e(nc.semaphore(f"sAx{i}")) for i in range(4)]
        sAmm = e(nc.semaphore("sAmm"))
        sAh = e(nc.semaphore("sAh"))
        sDma = [e(nc.semaphore(f"sDma{b}")) for b in range(NBLK)]
        sRd = [e(nc.semaphore(f"sRd{b}")) for b in range(NBLK)]
        sMg = e(nc.semaphore("sMg"))
        sFin = e(nc.semaphore("sFin"))

        # aliased views over scr (idx dead by the time these are used)
        accT = [scr[:, b * ACOLS * D:(b + 1) * ACOLS * D]
                .bitcast(dt.float16).rearrange("p (c d) -> p c d", d=D)
                for b in range(2)]
        # outT halves alias accT[1]'s bytes: block-1 data there is fully
        # consumed by the merges before the half-tails write outT
        outTh = [scr[:, ACOLS * D + h * (OCOLS // 2) * D:
                     ACOLS * D + (h + 1) * (OCOLS // 2) * D]
                 .bitcast(dt.float16).rearrange("p (c d) -> p c d", d=D)
                 for h in range(2)]

        def bcast(ap, reps):
            return bass.AP(ap.tensor, ap.offset, list(ap.ap) + [[0, reps]])

        with nc.Block() as block:

            @block.sync
            def _(sync: bass.BassEngine):
                sync.dma_start(WTs[:], t_WT[:]).then_inc(sLD, 16)
                sync.dma_start(degTs[:], t_degT[:]).then_inc(sLD, 16)
                sync.dma_start(degDs[:], t_degD[:]).then_inc(sLD, 16)
                sync.dma_start(bBCs[:], t_bBC[:]).then_inc(sLD, 16)
                sync.dma_start(scr[0:32, :], t_idx[:]
                               ).then_inc(sLD, 16)
                for it in range(AIT):
                    if it >= 4:
                        sync.wait_ge(sAmm, it - 3)
                    sync.dma_start(
                        xb[it % 4][:],
                        t_xT[:, it * 1024:(it + 1) * 1024],
                    ).then_inc(sAx[it % 4], 16)
                for b in range(NBLK):
                    sync.wait_ge(sDma[b], cnt_blk[b] * 16)
                for b in range(2):
                    src = bass.AP(t_acc[b], 0,
                                  [[ACOLS * 128, 128], [128, ACOLS],
                                   [1, D]])
                    sync.dma_start(accT[b % 2], src).then_inc(sRd[b], 16)
                # block 2 in two column halves into accT[0]; the half-0
                # merge/scale/bias/out overlap half 1's DMA
                sync.wait_ge(sMg, 1)
                HC = ACOLS // 2
                for h in range(2):
                    src = bass.AP(t_acc[2], h * HC * 128,
                                  [[ACOLS * 128, 128], [128, HC], [1, D]])
                    at = accT[0]
                    dsth = bass.AP(at.tensor, at.offset + h * HC * D,
                                   [list(at.ap[0]), [D, HC], [1, D]])
                    # distinct sems per half: completions may reorder
                    sync.dma_start(dsth, src).then_inc(sRd[h], 16)
                out3a = bass.AP(t_out, 0,
                                [[OCOLS * D, 128], [D, OCOLS // 2],
                                 [1, D]])
                sync.wait_ge(sMg, 7)
                sync.dma_start(out3a, outTh[0]).then_inc(sFin, 16)
                out3b = bass.AP(t_out, (OCOLS // 2) * D,
                                [[OCOLS * D, 128], [D, OCOLS // 2],
                                 [1, D]])
                sync.wait_ge(sMg, 11)
                sync.dma_start(out3b, outTh[1]).then_inc(sFin, 16)
                sync.wait_ge(sFin, 32)

            @block.tensor
            def _(tensor):
                tensor.wait_ge(sLD, 80)  # DMA completions may reorder
                for it in range(AIT):
                    tensor.wait_ge(sAx[it % 4], (it // 4 + 1) * 16)
                    if it >= 8:
                        tensor.wait_ge(sAh, it - 7)
                    for j in range(8):
                        ins = tensor.matmul(
                            psum[it % 8][:, j * D:(j + 1) * D],
                            xb[it % 4][:, j * 128:(j + 1) * 128],
                            WTs[:],
                            start=True, stop=True,
                        )
                    ins.then_inc(sAmm, 1)

            @block.vector
            def _(vector):
                vector.wait_ge(sLD, 80)  # DMA completions may reorder
                vector.reciprocal(dinvTs[:], degTs[:])
                vector.reciprocal(dinvDs[:], degDs[:]).then_inc(sDin, 1)
                vector.wait_ge(sDin, 2)
                for it in range(AIT):
                    vector.wait_ge(sAmm, it + 1)
                    ps3 = psum[it % 8][:].rearrange("p (c d) -> p c d", d=D)
                    dv = bcast(dinvTs[:, it * 8:(it + 1) * 8], D)
                    vector.tensor_tensor(
                        hp[:, it * 8:(it + 1) * 8, :], ps3, dv,
                        op=mybir.AluOpType.mult,
                    ).then_inc(sAh, 1)
                for b in range(2):
                    vector.wait_ge(sRd[b], 16)
                    at = accT[b % 2]
                    ev = bass.AP(at.tensor, at.offset,
                                 [list(at.ap[0]), [2 * D, OCOLS], [1, D]])
                    od = bass.AP(at.tensor, at.offset + D,
                                 [list(at.ap[0]), [2 * D, OCOLS], [1, D]])
                    if b == 0:
                        vector.tensor_tensor(
                            accm[:], ev, od, op=mybir.AluOpType.add
                        ).then_inc(sMg, 1)
                    else:
                        vector.tensor_tensor(
                            accm[:], accm[:], ev, op=mybir.AluOpType.add
                        ).then_inc(sMg, 1)
                        vector.tensor_tensor(
                            accm[:], accm[:], od, op=mybir.AluOpType.add
                        ).then_inc(sMg, 1)
                at = accT[0]
                OH = OCOLS // 2
                for h in range(2):
                    vector.wait_ge(sRd[h], 32)
                    co = h * OH
                    ev = bass.AP(at.tensor, at.offset + 2 * co * D,
                                 [list(at.ap[0]), [2 * D, OH], [1, D]])
                    od = bass.AP(at.tensor, at.offset + 2 * co * D + D,
                                 [list(at.ap[0]), [2 * D, OH], [1, D]])
                    ah = accm[:, co:co + OH, :]
                    vector.tensor_tensor(
                        ah, ah, ev, op=mybir.AluOpType.add
                    ).then_inc(sMg, 1)
                    vector.tensor_tensor(
                        ah, ah, od, op=mybir.AluOpType.add
                    ).then_inc(sMg, 1)
                    dvD = bcast(dinvD16[:, co:co + OH], D)
                    vector.tensor_tensor(
                        ah, ah, dvD, op=mybir.AluOpType.mult
                    ).then_inc(sMg, 1)
                    apb = bBCs[:]
                    bb = bass.AP(apb.tensor, apb.offset,
                                 [list(apb.ap[0]), [0, OH],
                                  list(apb.ap[1])])
                    vector.tensor_tensor(
                        outTh[h], ah, bb, op=mybir.AluOpType.add
                    ).then_inc(sMg, 1)

            @block.scalar
            def _(scalar):
                scalar.wait_ge(sDin, 1)
                scalar.activation(dinvTs[:], dinvTs[:],
                                  mybir.ActivationFunctionType.Sqrt)
                scalar.activation(dinvDs[:], dinvDs[:],
                                  mybir.ActivationFunctionType.Sqrt)
                scalar.activation(dinvD16[:], dinvDs[:],
                                  mybir.ActivationFunctionType.Copy
                                  ).then_inc(sDin, 1)

            @block.gpsimd
            def _(gpsimd: bass.BassGpSimd):
                gpsimd.load_library(mlp)
                gpsimd.wait_ge(sLD, 80)
                for i, (a, w, ioff, treq) in enumerate(sched):
                    b = i % NBLK
                    gpsimd.wait_ge(sAh, treq)
                    if i >= NBLK:
                        gpsimd.wait_ge(sDma[b], (i // NBLK) * 16)
                    out_ap = bass.AP(t_acc[b], 0,
                                     [[128, ACCROWS], [1, D]])
                    gpsimd.dma_scatter_add(
                        out_ap,
                        hp[:, a:a + w, :],
                        scr[:, ioff:ioff + 8 * w],
                        128 * w, 128 * w, D, elem_step=128,
                    ).then_inc(sDma[b], 16)

        nc.compile()
    return nc


def kernel(x, edge_index, W, b):
    x = np.asarray(x, dtype=np.float32)
    edge_index = np.asarray(edge_index)
    W = np.asarray(W, dtype=np.float32)
    b = np.asarray(b, dtype=np.float32)

    src = np.asarray(edge_index[0], dtype=np.int64)
    dst = np.asarray(edge_index[1], dtype=np.int64)

    deg_glob = (np.bincount(dst, minlength=N) + 1.0).astype(np.float32)

    cores = []
    degs = []
    for c in range(NCORES):
        m = (dst >= c * SHARD) & (dst < (c + 1) * SHARD)
        sg = np.concatenate([src[m], np.arange(c * SHARD, (c + 1) * SHARD)])
        dl = np.concatenate([dst[m] - c * SHARD, np.arange(SHARD)])
        cores.append((sg, dl))
        degs.append(np.bincount(sg, minlength=N))
    maxdeg = max(int(d.max()) for d in degs)
    K = np.array([max(int((d >= r).sum()) for d in degs)
                  for r in range(1, maxdeg + 1)])

    calls, KCOLS = _build_geometry(K)
    solved = [_solve_core(*cores[c], K, calls, seed=c * 7 + 1)
              for c in range(NCORES)]

    treqs = [-(-(a + w) // 8) for (a, w, lev) in calls]
    cum = np.cumsum([0] + [8 * w for (a, w, lev) in calls])
    order_c = sorted(range(len(calls)), key=lambda i: treqs[i])
    sched = [(calls[i][0], calls[i][1], int(cum[i]), treqs[i])
             for i in order_c]
    IDXC = max(int(cum[-1]), 2 * ACOLS * D)

    WT16 = np.ascontiguousarray(W.T).astype(np.float16)
    bBC = np.ascontiguousarray(np.broadcast_to(b, (128, D))
                               ).astype(np.float16)

    in_maps = []
    for c in range(NCORES):
        order, per_call = solved[c]
        sel = order[:KCOLS * 128]
        xT = np.ascontiguousarray(x[sel].astype(np.float16).T)
        degT = np.ascontiguousarray(
            deg_glob[sel].reshape(KCOLS, 128).T).astype(np.float32)
        degD_flat = np.ones(OUTROWS, np.float32)
        degD_flat[:SHARD] = deg_glob[c * SHARD:(c + 1) * SHARD]
        degD = np.ascontiguousarray(degD_flat.reshape(128, OCOLS))
        idx_tbl = np.zeros((32, IDXC), np.int16)
        for i, arr in enumerate(per_call):
            idx_tbl[:, int(cum[i]):int(cum[i + 1])] = _wrap16(arr)
        in_maps.append({
            "xT": xT, "WT": WT16, "degT": degT, "degD": degD,
            "bBC": bBC, "idx": idx_tbl,
        })

    nc = _build_program(KCOLS, IDXC, sched)
    global LAST_NC
    LAST_NC = nc
    res = bass_utils.run_bass_kernel_spmd(nc, in_maps,
                                          core_ids=list(range(NCORES)))
    out = np.empty((N, D), np.float32)
    for c in range(NCORES):
        o16 = np.asarray(res.results[c]["out_s"])
        out[c * SHARD:(c + 1) * SHARD] = o16[:SHARD].astype(np.float32)
    return out
